# revision 18
# baseline (speedup 1.0000x reference)
"""Additive (Bahdanau) attention kernel for Trainium2, SPMD over 8 NeuronCores.

score[b,l,k] = sum_a w3[a] * tanh(qp[b,l,a] + kp[b,k,a]);  masked softmax over k
  qp = Q @ W1^T, kp = K @ W2^T

Sharding: data-parallel over batch B=8 (one batch per core), weights replicated.

Algorithm: sine-ridge decomposition, tanh(z) ~ c0 + c1 z + sum b_m sin(m w0 z)
over harmonic modes {1,2,3,4,6}; each sine splits by angle addition into two
separable products, making the score a sum of rank-structured bf16 matmuls with
contraction over a.  Terms depending only on the query side drop out under
softmax.  |w0 x| < pi for all projected values, so mode-1 sin comes straight
from ACT; cos via the half-angle identity c1 = 1 - 2 sin^2(w0 x/2); higher
modes via exact bf16 Chebyshev recurrences (tensor_tensor against doubled
tensors; STT is 1x on DVE, and GPSIMD streaming ops both run slowly and lock
the SBUF port shared with DVE, so neither is used).  Inputs are pre-cast to
bf16 and the mask pre-converted to an additive bias on the host.  Each input
tensor is split across the sync/scalar/gpsimd DMA queues (a single queue
sustains only ~100 GB/s).  Dummy matmuls keep the PE clock-gate warm while the
ladder runs.
"""

import sys

import numpy as np

if "/opt/trn_rl_repo" not in sys.path:
    sys.path.insert(0, "/opt/trn_rl_repo")

B, LQ, LK, D, A = 8, 256, 256, 512, 256
N_CORES = 8

W0 = 0.5076930427551914
C1LIN = 0.16160049086133022
BS = (0.5685581803112249, 0.22546011634371437, 0.07807929846270594,
      0.06083552909344006, 0.02174126576121101)

_cached_nc = None


def _build():
    from contextlib import ExitStack

    import concourse.mybir as mybir
    from concourse import tile
    from concourse.bacc import Bacc

    FP = mybir.dt.float32
    BF = mybir.dt.bfloat16
    F16 = mybir.dt.float16
    Act = mybir.ActivationFunctionType
    Alu = mybir.AluOpType

    nc = Bacc()
    # bf16 payloads packed as fp32 (DMA queues are element-rate limited)
    KWd = nc.declare_dram_parameter("KW", [128, 8, 128], FP, isOutput=False)
    QWd = nc.declare_dram_parameter("QW", [128, 8, 128], FP, isOutput=False)
    MBd = nc.declare_dram_parameter("mbias", [128, 2, 256], FP, isOutput=False)
    W3Bd = nc.declare_dram_parameter("w3b", [128, 2, 8], FP, isOutput=False)
    W3Zd = nc.declare_dram_parameter("w3z", [128, 2, 64], FP, isOutput=False)
    Od = nc.declare_dram_parameter("out", [LQ, LK], FP, isOutput=True)

    with tile.TileContext(nc) as tc:
        with ExitStack() as ctx:
            const = ctx.enter_context(tc.tile_pool(name="const", bufs=1))
            io = ctx.enter_context(tc.tile_pool(name="io", bufs=1))
            fac = ctx.enter_context(tc.tile_pool(name="fac", bufs=1))
            smx = ctx.enter_context(tc.tile_pool(name="smx", bufs=2))
            ppj = ctx.enter_context(tc.tile_pool(name="ppj", bufs=1, space="PSUM"))
            psc = ctx.enter_context(tc.tile_pool(name="psc", bufs=1, space="PSUM"))
            pwu = ctx.enter_context(tc.tile_pool(name="pwu", bufs=1, space="PSUM"))

            dumb1 = const.tile([128, 1], FP)
            nc.vector.memset(dumb1[:], 0.25)
            junk = const.tile([128, 128], BF)
            nc.vector.memset(junk[:], 0.5)
            dumb2 = const.tile([128, 1], FP)

            # input DMAs, each tensor split across queues (sync / scalar /
            # gpsimd); k-side parts first, mask bias last
            kw = io.tile([128, 8, 128], FP)
            qw = io.tile([128, 8, 128], FP)
            mb = io.tile([128, 2, 256], FP)
            w3b = const.tile([128, 2, 8], FP)
            w3z = const.tile([128, 2, 64], FP)
            nc.sync.dma_start(kw[:, 0:4, :], KWd[:, 0:4, :])      # K
            nc.scalar.dma_start(kw[:, 4:8, :], KWd[:, 4:8, :])    # W2
            nc.gpsimd.dma_start(qw[:, 0:4, :], QWd[:, 0:4, :])    # Q
            nc.sync.dma_start(qw[:, 4:8, :], QWd[:, 4:8, :])      # W1
            nc.gpsimd.dma_start(w3b[:], W3Bd[:])
            nc.gpsimd.dma_start(w3z[:], W3Zd[:])
            nc.scalar.dma_start(mb[:], MBd[:])
            # trig-table preload trigger (after the scalar-queue DMA issues)
            nc.scalar.activation(dumb2[:], dumb1[:], Act.Sin)

            # PE warmup during DMA wait
            pdum = pwu.tile([128, 128], FP)
            for _ in range(30):
                nc.tensor.matmul(pdum[:], junk[:], junk[:], start=True,
                                 stop=True)

            # projections (bf16): pqk [a(128), side(q=0,k=1), at, l/k]
            pqk = ppj.tile([128, 2, 2, 256], FP)

            def project(side, xw):
                for at in range(2):
                    for db in range(4):
                        nc.tensor.matmul(
                            pqk[:, side, at, :],
                            xw[:, 4 + db, at * 64:(at + 1) * 64].bitcast(BF),
                            xw[:, db, :].bitcast(BF),
                            start=(db == 0), stop=(db == 3),
                        )

            project(1, kw)   # k first
            project(0, qw)

            # keep PE warm while the ladder runs (junk, no deps on data)
            for _ in range(14):
                nc.tensor.matmul(pdum[:], junk[:], junk[:], start=True,
                                 stop=True)

            def side_tiles(prefix):
                t = {}
                for nm in ("s1", "c1", "d1", "s2", "c2t", "c2", "s3t", "s3",
                           "c3t", "c3", "d2", "d3", "s4", "c4t", "c4", "s6",
                           "c6t", "c6"):
                    t[nm] = fac.tile([128, 2, 256], BF, name=f"{prefix}{nm}")
                return t

            K_, Q_ = side_tiles("k"), side_tiles("q")
            hs_k = fac.tile([128, 2, 256], F16, name="hsk")
            hs_q = fac.tile([128, 2, 256], F16, name="hsq")
            G = {}
            for nm in ("s1", "c1", "s2", "c2", "s3", "c3", "s4", "c4", "s6",
                       "c6"):
                G[nm] = fac.tile([128, 2, 256], BF, name=f"g{nm}")
            kp_bf = fac.tile([128, 2, 256], BF, name="kpbf")

            def sins(side, s1, hs):
                src = pqk[:, side, :, :]
                nc.scalar.activation(hs[:], src, Act.Sin, scale=float(W0 / 2))
                nc.scalar.activation(s1[:], src, Act.Sin, scale=float(W0))

            sins(1, K_["s1"], hs_k)
            nc.scalar.activation(kp_bf[:], pqk[:, 1, :, :], Act.Copy)

            def ladder(T, hs, exact_c46=True):
                v = nc.vector
                v.tensor_tensor(T["c1"][:], hs[:], hs[:], op=Alu.mult)
                v.tensor_scalar(T["c1"][:], T["c1"][:], -2.0, 1.0,
                                op0=Alu.mult, op1=Alu.add)
                v.tensor_scalar(T["d1"][:], T["c1"][:], 2.0, None,
                                op0=Alu.mult)
                v.tensor_tensor(T["s2"][:], T["s1"][:], T["d1"][:],
                                op=Alu.mult)
                v.tensor_tensor(T["c2t"][:], T["c1"][:], T["d1"][:],
                                op=Alu.mult)
                v.tensor_scalar(T["c2"][:], T["c2t"][:], -1.0, None,
                                op0=Alu.add)
                v.tensor_tensor(T["s3t"][:], T["s2"][:], T["d1"][:],
                                op=Alu.mult)
                v.tensor_tensor(T["s3"][:], T["s3t"][:], T["s1"][:],
                                op=Alu.subtract)
                v.tensor_tensor(T["c3t"][:], T["c2"][:], T["d1"][:],
                                op=Alu.mult)
                v.tensor_tensor(T["c3"][:], T["c3t"][:], T["c1"][:],
                                op=Alu.subtract)
                v.tensor_scalar(T["d2"][:], T["c2"][:], 2.0, None,
                                op0=Alu.mult)
                v.tensor_tensor(T["s4"][:], T["s2"][:], T["d2"][:],
                                op=Alu.mult)
                v.tensor_tensor(T["c4t"][:], T["c2"][:], T["d2"][:],
                                op=Alu.mult)
                if exact_c46:
                    v.tensor_scalar(T["c4"][:], T["c4t"][:], -1.0, None,
                                    op0=Alu.add)
                v.tensor_scalar(T["d3"][:], T["c3"][:], 2.0, None,
                                op0=Alu.mult)
                v.tensor_tensor(T["s6"][:], T["s3"][:], T["d3"][:],
                                op=Alu.mult)
                v.tensor_tensor(T["c6t"][:], T["c3"][:], T["d3"][:],
                                op=Alu.mult)
                if exact_c46:
                    v.tensor_scalar(T["c6"][:], T["c6t"][:], -1.0, None,
                                    op0=Alu.add)

            ladder(K_, hs_k, exact_c46=False)
            sins(0, Q_["s1"], hs_q)

            # k-side folds: c-factors fused at-split ts on DVE (the -1 of
            # c4/c6 is absorbed into the fold), s-factors at-split on ACT
            for m, nm in ((0, "c1"), (1, "c2")):
                for at in range(2):
                    nc.scalar.activation(
                        G[nm][:, at, :], K_[nm][:, at, :], Act.Identity,
                        bias=0.0, scale=w3b[:, at, m:m + 1])
            for m, nm, src, bias in ((2, "c3", "c3", None),
                                     (3, "c4", "c4t", -1.0),
                                     (4, "c6", "c6t", -1.0)):
                for at in range(2):
                    if bias is None:
                        nc.vector.tensor_scalar(
                            G[nm][:, at, :], K_[src][:, at, :],
                            w3b[:, at, m:m + 1], None, op0=Alu.mult)
                    else:
                        nc.vector.tensor_scalar(
                            G[nm][:, at, :], K_[src][:, at, :], bias,
                            w3b[:, at, m:m + 1], op0=Alu.add, op1=Alu.mult)
            for m, nm in enumerate(("s1", "s2", "s3", "s4", "s6")):
                for at in range(2):
                    nc.scalar.activation(
                        G[nm][:, at, :], K_[nm][:, at, :], Act.Identity,
                        bias=0.0, scale=w3b[:, at, m:m + 1])

            ladder(Q_, hs_q, exact_c46=True)

            # --- score matmuls (bf16) --------------------------------------
            sc = [psc.tile([128, 256], FP, name=f"sc{i}") for i in range(2)]
            n_per_lc = (1 + 2 * 5) * 2
            cnt = [0, 0]

            def score_mm(lc, lhsT, rhs):
                nc.tensor.matmul(sc[lc][:], lhsT, rhs,
                                 start=(cnt[lc] == 0),
                                 stop=(cnt[lc] == n_per_lc - 1))
                cnt[lc] += 1

            for at in range(2):
                for lc in range(2):
                    score_mm(lc, w3z[:, at, :].bitcast(BF),
                             kp_bf[:, at, :])
            qnames = ["s1", "c1", "s2", "c2", "s3", "c3", "s4", "c4", "s6",
                      "c6"]
            gnames = ["c1", "s1", "c2", "s2", "c3", "s3", "c4", "s4", "c6",
                      "s6"]
            for i, (qn, gn) in enumerate(zip(qnames, gnames)):
                if i < 8:
                    for at in range(2):
                        for lc in range(2):
                            sl = slice(lc * 128, (lc + 1) * 128)
                            score_mm(lc, Q_[qn][:, at, sl], G[gn][:, at, :])
                else:
                    # close lc0's accumulation first so its softmax overlaps
                    for lc in range(2):
                        for at in range(2):
                            sl = slice(lc * 128, (lc + 1) * 128)
                            score_mm(lc, Q_[qn][:, at, sl], G[gn][:, at, :])

            # --- masked softmax over k -------------------------------------
            for lc in range(2):
                masked = smx.tile([128, 256], FP)
                nc.vector.tensor_add(masked[:], sc[lc][:], mb[:, lc, :])
                e = smx.tile([128, 256], FP)
                sums = smx.tile([128, 1], FP)
                nc.scalar.activation(e[:], masked[:], Act.Exp,
                                     bias=0.0, scale=1.0, accum_out=sums[:])
                recip = smx.tile([128, 1], FP)
                nc.vector.reciprocal(recip[:], sums[:])
                outt = smx.tile([128, 256], FP)
                nc.vector.tensor_scalar_mul(outt[:], e[:], recip[:])
                eng = nc.sync if lc == 0 else nc.scalar
                eng.dma_start(Od[lc * 128:(lc + 1) * 128, :], outt[:])

    nc.compile()
    return nc


def _get_nc():
    global _cached_nc
    if _cached_nc is None:
        _cached_nc = _build()
    return _cached_nc


def _pack_side(xT, wT, bf):
    xr = xT.reshape(4, 128, -1).transpose(1, 0, 2)
    wr = wT.reshape(4, 128, -1).transpose(1, 0, 2)
    return np.ascontiguousarray(
        np.concatenate([xr, wr], axis=1)).astype(bf).view(np.float32)


def _make_in_maps(inputs):
    import ml_dtypes
    bf = ml_dtypes.bfloat16

    Q = np.asarray(inputs["Q"], dtype=np.float32).reshape(B, LQ, D)
    K = np.asarray(inputs["K"], dtype=np.float32).reshape(B, LK, D)
    mask = np.asarray(inputs["mask"], dtype=np.int32)
    W1 = np.asarray(inputs["W1"], dtype=np.float32)
    W2 = np.asarray(inputs["W2"], dtype=np.float32)
    w3 = np.asarray(inputs["w3"], dtype=np.float64)

    W1T = np.ascontiguousarray(W1.T)
    W2T = np.ascontiguousarray(W2.T)
    w3at = w3.reshape(2, 128).T
    cols = [BS[0], BS[1], BS[2], BS[3], BS[4], 0.0, 0.0, 0.0]
    w3b = np.ascontiguousarray(
        (w3at[:, :, None] * np.asarray(cols)[None, None, :]),
        dtype=np.float32)
    w3z = np.ascontiguousarray(
        np.broadcast_to((w3at * C1LIN)[:, :, None], (128, 2, 128)),
        dtype=bf).view(np.float32)
    mbias = np.where(mask == 0, np.float32(-1.0e15), np.float32(0.0))

    maps = []
    for i in range(B):
        maps.append(dict(
            KW=_pack_side(np.ascontiguousarray(K[i].T), W2T, bf),
            QW=_pack_side(np.ascontiguousarray(Q[i].T), W1T, bf),
            mbias=np.ascontiguousarray(
                mbias[i].reshape(2, 128, 256).transpose(1, 0, 2)),
            w3b=w3b, w3z=w3z,
        ))
    return maps


def _run(inputs, trace=False, tmpdir=None):
    from concourse.bass_utils import run_bass_kernel_spmd

    nc = _get_nc()
    in_maps = _make_in_maps(inputs)
    res = run_bass_kernel_spmd(
        nc, in_maps, list(range(N_CORES)), trace=trace, tmpdir=tmpdir
    )
    out = np.stack([res.results[i]["out"] for i in range(N_CORES)], axis=0)
    return out, res


def kernel(**inputs) -> np.ndarray:
    out, _ = _run(inputs, trace=False)
    return out
